# revision 19
# baseline (speedup 1.0000x reference)
"""AttentionBlock (GroupNorm + 4-head self-attention + proj + residual) on 8 trn2 cores.

v3: HAM-warm PE + 2-way row-tiled fp8 S matmuls + bf16 AV + fast reciprocal.

Sharding: data-parallel over batch. B=16 -> 2 batches per core. Weights replicated.

Key points vs v2:
  - All attention matmuls non-DR: S = k^T q per head with K=64 in a 64-row
    strip (tile_position (64*(h%2), 0)); adjacent MMs alternate strips so
    2 heads' S MMs run concurrently in the PE array (measured 113ns/MM).
  - exp(S/8)/16 -> bf16 on ACT (native Exp) or DVE (Schraudolph int16),
    granule [128,2,512]; AV is bf16 with lhsT = [V^T | ones] (M=65).
  - Softmax recip: sums row -> DRAM -> [64,16] reshape -> DVE recip (fast,
    partition-parallel) -> DRAM -> row-broadcast [64,512]; Pool multiplies.
  - PE pre-warm dummy matmuls during the initial x DMA + fillers during
    GroupNorm stalls keep the HAM clock-gate at K=8/8 (2.4 GHz).
  - PSUM: tag "sp" 2x[128,2,512] (4 banks) + tag "pav" 4x[65/128,512]
    (4 banks) = 8 banks, time-multiplexed across GN/QKV/attention/proj.
"""
import numpy as np
from contextlib import ExitStack

import concourse.bass as bass
import concourse.bacc as bacc
import concourse.tile as tile
from concourse import mybir
from concourse import bass_utils

F32 = mybir.dt.float32
F32R = mybir.dt.float32r
BF16 = mybir.dt.bfloat16
F8 = mybir.dt.float8e4
I32 = mybir.dt.int32
I16 = mybir.dt.int16
AF = mybir.ActivationFunctionType
OP = mybir.AluOpType

B, C, H, W = 16, 256, 32, 32
N = H * W            # 1024
NH, D = 4, 64
G, GS = 32, 8        # groups, channels per group
EPS = 1e-5
NCORES = 8
BL = B // NCORES     # 2 batches per core
NCH = C // 128       # 2 channel chunks
NMC = N // 128       # 8 m-chunks
NMCP = NMC // 2      # 4 m-chunk pairs
INV_GSZ = 1.0 / (GS * N)  # 1/8192
ATT_SCALE = 0.125         # 1/sqrt(64)
LN16 = float(np.log(16.0))
# Schraudolph exp: bitcast(int16(A*x + B)) as bf16 ~ exp(x); x = S*0.125 - ln16
A_SCH = (2 ** 7) / np.log(2.0)
SCH_MUL = float(A_SCH * ATT_SCALE)
SCH_ADD = float(127 * 2 ** 7 - 366000.0 / 2 ** 16 - A_SCH * LN16)

_CACHE = {}


def _build_nc(reps=1):
    nc = bacc.Bacc()
    x = nc.declare_dram_parameter("x", [BL, C, N], F32R, isOutput=False)
    wqk8 = nc.declare_dram_parameter("wqk8", [128, NCH, 4, 128], F8, isOutput=False)
    wv8 = nc.declare_dram_parameter("wv8", [128, NCH, C], F8, isOutput=False)
    wpT4 = nc.declare_dram_parameter("wpT4", [NH, D, C], F32R, isOutput=False)
    qkb = nc.declare_dram_parameter("qkb", [4, 128], F32, isOutput=False)
    pbv = nc.declare_dram_parameter("pbv", [C], F32, isOutput=False)
    gamma = nc.declare_dram_parameter("gamma", [C], F32, isOutput=False)
    beta = nc.declare_dram_parameter("beta", [C], F32, isOutput=False)
    sel = nc.declare_dram_parameter("sel", [NCH, 128, G], F32R, isOutput=False)
    sel_exp = nc.declare_dram_parameter("sel_exp", [G, NCH, 128], F32, isOutput=False)
    wzd = nc.declare_dram_parameter("wzd", [32, 1024], F8, isOutput=False)
    out = nc.declare_dram_parameter("out", [BL, C, N], F32, isOutput=True)

    sums_dram = nc.dram_tensor("sums_dram", [BL, 2, NH, 512], F32)
    recip_dram = nc.dram_tensor("recip_dram", [BL, 2, NH, 512], F32)

    def bcast_ap(dram_row_ap, parts):
        return bass.AP(tensor=dram_row_ap.tensor, offset=dram_row_ap.offset,
                       ap=[[0, parts]] + [list(d) for d in dram_row_ap.ap])

    with tile.TileContext(nc) as tc, ExitStack() as ctx:
        if reps > 1:
            ctx.enter_context(tc.For_i(0, reps, 1, hint_engines=(
                mybir.EngineType.PE, mybir.EngineType.Activation,
                mybir.EngineType.DVE, mybir.EngineType.SP,
                mybir.EngineType.Pool)))
        const = ctx.enter_context(tc.tile_pool(name="const", bufs=1))
        xpool = ctx.enter_context(tc.tile_pool(name="xpool", bufs=2))
        sqpool = ctx.enter_context(tc.tile_pool(name="sqpool", bufs=2))
        hpool = ctx.enter_context(tc.tile_pool(name="hpool", bufs=2))
        qkpool = ctx.enter_context(tc.tile_pool(name="qkpool", bufs=1))
        vfpool = ctx.enter_context(tc.tile_pool(name="vfpool", bufs=2))
        tiny = ctx.enter_context(tc.tile_pool(name="tiny", bufs=2))
        abpool = ctx.enter_context(tc.tile_pool(name="abpool", bufs=4))
        etpool = ctx.enter_context(tc.tile_pool(name="etpool", bufs=6))
        avpool = ctx.enter_context(tc.tile_pool(name="avpool", bufs=1))
        sspool = ctx.enter_context(tc.tile_pool(name="sspool", bufs=4))
        bcpool = ctx.enter_context(tc.tile_pool(name="bcpool", bufs=4))
        outpool = ctx.enter_context(tc.tile_pool(name="outpool", bufs=3))
        ps = ctx.enter_context(tc.tile_pool(name="ps", bufs=2, space="PSUM"))

        # ---- warm operand + selexp first, then x (critical path) ----
        wz = const.tile([32, 1024], F8, tag="wz")
        nc.sync.dma_start(out=wz, in_=wzd[:, :])
        selexp_t = const.tile([G, NCH * 128], F32, tag="selexp")
        nc.sync.dma_start(out=selexp_t, in_=sel_exp.rearrange("g c p -> g (c p)"))
        xt = [None] * BL
        for b in range(BL):
            t = xpool.tile([128, NCH, N], F32R, tag="xt")
            nc.sync.dma_start(out=t, in_=x[b].rearrange("(c p) n -> p c n", p=128))
            xt[b] = t
        sel_t, gam_t, bet_t, pbv_t = [], [], [], []
        for c in range(NCH):
            t = const.tile([128, G], F32R, tag=f"sel{c}")
            nc.sync.dma_start(out=t, in_=sel[c, :, :])
            sel_t.append(t)
            t = const.tile([128, 1], F32, tag=f"gam{c}")
            nc.sync.dma_start(out=t, in_=gamma[c * 128:(c + 1) * 128].rearrange("(p o) -> p o", o=1))
            gam_t.append(t)
            t = const.tile([128, 1], F32, tag=f"bet{c}")
            nc.sync.dma_start(out=t, in_=beta[c * 128:(c + 1) * 128].rearrange("(p o) -> p o", o=1))
            bet_t.append(t)
            t = const.tile([128, 1], F32, tag=f"pbv{c}")
            nc.sync.dma_start(out=t, in_=pbv[c * 128:(c + 1) * 128].rearrange("(p o) -> p o", o=1))
            pbv_t.append(t)
        wqk8_t = const.tile([128, NCH, 4, 128], F8, tag="wqk8")
        nc.sync.dma_start(out=wqk8_t, in_=wqk8[:, :, :, :])
        wv8_t = const.tile([128, NCH, C], F8, tag="wv8")
        nc.sync.dma_start(out=wv8_t, in_=wv8[:, :, :])
        wpT_t = []
        for h in range(NH):
            t = const.tile([D, C], F32R, tag=f"wpT{h}")
            nc.sync.dma_start(out=t, in_=wpT4[h, :, :])
            wpT_t.append(t)
        qkb_t = []
        for j in range(4):
            t = const.tile([128, 1], F32, tag=f"qkb{j}")
            nc.sync.dma_start(out=t, in_=qkb[j, :].rearrange("(p o) -> p o", o=1))
            qkb_t.append(t)
        expb = const.tile([128, 1], F32, tag="expb")
        nc.vector.memset(expb, -LN16)
        scr1 = const.tile([1, 1], F32, tag="scr1")
        nc.scalar.activation(out=scr1, in_=expb[0:1, :], func=AF.Exp)
        def warm_mm(n):
            for _ in range(n):
                pw = ps.tile([128, 512], F32, tag="pav", bufs=4)
                nc.tensor.matmul(out=pw, lhsT=wz[:, 0:128],
                                 rhs=wz[:, 128:640],
                                 start=True, stop=True, tile_position=(0, 0))

        warm_mm(12)

        # ---- phase 1: groupnorm -> h8 (fp8 normalized x) ----
        # bn_stats on x directly: per-partition (count, mean, count*var) for
        # even/odd halves of each 512-slice; combine to per-partition
        # (sum-ish, sumsq-ish), then one tiny sel matmul per chunk reduces
        # the 8-channel groups. stb cols as before; INV scale = 1/32.
        stb_all = [None] * BL

        def emit_gn_stats(b):
            sbn = tiny.tile([128, NCH, 2, 6], F32, tag="sbn")
            for c in range(NCH):
                for n2 in range(2):
                    nc.vector.bn_stats(
                        out=sbn[:, c, n2, :],
                        in_=xt[b][:, c, n2 * 512:(n2 + 1) * 512].bitcast(F32))
            sfx = tiny.tile([128, NCH, 2, 3], F32, tag="sfx")
            nc.vector.tensor_tensor(out=sfx[:, :, :, 0], in0=sbn[:, :, :, 1],
                                    in1=sbn[:, :, :, 4], op=OP.add)
            nc.vector.tensor_tensor(out=sfx[:, :, :, 1], in0=sbn[:, :, :, 2],
                                    in1=sbn[:, :, :, 5], op=OP.add)
            nc.vector.tensor_tensor(out=sbn[:, :, :, 0], in0=sbn[:, :, :, 1],
                                    in1=sbn[:, :, :, 1], op=OP.mult)
            nc.vector.tensor_tensor(out=sbn[:, :, :, 3], in0=sbn[:, :, :, 4],
                                    in1=sbn[:, :, :, 4], op=OP.mult)
            nc.vector.tensor_tensor(out=sfx[:, :, :, 2], in0=sbn[:, :, :, 0],
                                    in1=sbn[:, :, :, 3], op=OP.add)
            rhsb = tiny.tile([128, NCH, 2], F32, tag="rhsb")
            nc.vector.tensor_tensor(out=rhsb[:, :, 0], in0=sfx[:, :, 0, 0],
                                    in1=sfx[:, :, 1, 0], op=OP.add)
            nc.vector.tensor_tensor(out=sfx[:, :, 0, 1], in0=sfx[:, :, 0, 1],
                                    in1=sfx[:, :, 1, 1], op=OP.add)
            nc.vector.tensor_tensor(out=sfx[:, :, 0, 2], in0=sfx[:, :, 0, 2],
                                    in1=sfx[:, :, 1, 2], op=OP.add)
            nc.vector.tensor_scalar_mul(sfx[:, :, 0, 1], sfx[:, :, 0, 1],
                                        1.0 / 256.0)
            nc.vector.tensor_tensor(out=rhsb[:, :, 1], in0=sfx[:, :, 0, 1],
                                    in1=sfx[:, :, 0, 2], op=OP.add)
            stat_ps = ps.tile([G, 2], F32, tag="pav", bufs=4)
            for c in range(NCH):
                nc.tensor.matmul(out=stat_ps, lhsT=sel_t[c].bitcast(F32),
                                 rhs=rhsb[:, c, :],
                                 start=(c == 0), stop=(c == NCH - 1))
            # stats cols: 0=s 1=q 2=mean 3=rstd 4=msq 5=m2 6=var 7=lnv
            stb = tiny.tile([G, 8], F32, tag="stats")
            nc.vector.tensor_copy(out=stb[:, 0:2], in_=stat_ps)
            nc.vector.tensor_scalar_mul(stb[:, 2:3], stb[:, 0:1], 1.0 / 32.0)
            nc.vector.tensor_scalar_mul(stb[:, 4:5], stb[:, 1:2], 1.0 / 32.0)
            nc.vector.tensor_tensor(out=stb[:, 5:6], in0=stb[:, 2:3], in1=stb[:, 2:3], op=OP.mult)
            nc.vector.tensor_tensor(out=stb[:, 6:7], in0=stb[:, 4:5], in1=stb[:, 5:6], op=OP.subtract)
            # rstd via bit-trick rsqrt + 1 Newton step (all DVE)
            nc.vector.tensor_scalar_add(stb[:, 7:8], stb[:, 6:7], EPS)
            i32v = stb.bitcast(I32)
            nc.vector.tensor_scalar(out=i32v[:, 0:1], in0=i32v[:, 7:8],
                                    scalar1=1, scalar2=-1,
                                    op0=OP.logical_shift_right, op1=OP.bitwise_xor)
            nc.vector.tensor_scalar(out=i32v[:, 1:2], in0=i32v[:, 0:1],
                                    scalar1=0x5f3759e0, scalar2=None, op0=OP.add)
            nc.vector.tensor_tensor(out=stb[:, 0:1], in0=stb[:, 7:8], in1=stb[:, 1:2], op=OP.mult)
            nc.vector.tensor_tensor(out=stb[:, 0:1], in0=stb[:, 0:1], in1=stb[:, 1:2], op=OP.mult)
            nc.vector.tensor_scalar(out=stb[:, 0:1], in0=stb[:, 0:1],
                                    scalar1=-0.5, scalar2=1.5,
                                    op0=OP.mult, op1=OP.add)
            nc.vector.tensor_tensor(out=stb[:, 3:4], in0=stb[:, 1:2], in1=stb[:, 0:1], op=OP.mult)
            stb_all[b] = stb

        h8 = [None] * BL

        def emit_gn_h8(b):
            stb = stb_all[b]
            h8b = hpool.tile([128, NCH, N], F8, tag="h8")
            h8[b] = h8b
            for c in range(NCH):
                e_ps = ps.tile([128, 2], F32, tag="pav", bufs=4)
                nc.tensor.matmul(
                    out=e_ps, lhsT=selexp_t[:, c * 128:(c + 1) * 128],
                    rhs=stb[:, 2:4], start=True, stop=True)
                ab = abpool.tile([128, 3], F32, tag="ab")
                abe = abpool.tile([128, 2], F32, tag="abe")
                nc.vector.tensor_copy(out=abe, in_=e_ps)
                nc.vector.tensor_tensor(out=ab[:, 0:1], in0=abe[:, 1:2], in1=gam_t[c], op=OP.mult)
                nc.vector.tensor_tensor(out=ab[:, 2:3], in0=abe[:, 0:1], in1=ab[:, 0:1], op=OP.mult)
                nc.vector.tensor_tensor(out=ab[:, 1:2], in0=bet_t[c], in1=ab[:, 2:3], op=OP.subtract)
                nc.gpsimd.tensor_scalar(out=h8b[:, c, :], in0=xt[b][:, c, :].bitcast(F32),
                                         scalar1=ab[:, 0:1], scalar2=ab[:, 1:2],
                                         op0=OP.mult, op1=OP.add)

        emit_gn_stats(0)
        emit_gn_h8(0)
        warm_mm(4)
        emit_gn_stats(1)
        emit_gn_h8(1)
        warm_mm(6)

        # ---- phase 2: qkv ----
        # qk8[b][j]: [128, N] fp8; j=0: q heads01, j=1: q heads23, 2/3: k
        qk8 = [[None] * 4 for _ in range(BL)]
        vtf = [None] * BL

        def emit_qkv(b):
            for j in range(4):
                pj = ps.tile([128, N], F32, tag="sp", bufs=2)
                for nh2 in range(2):
                    sl = slice(nh2 * 512, (nh2 + 1) * 512)
                    for c in range(NCH):
                        nc.tensor.matmul(
                            out=pj[:, sl],
                            lhsT=wqk8_t[:, c, j, :],
                            rhs=h8[b][:, c, sl],
                            start=(c == 0), stop=(c == NCH - 1))
                qt = qkpool.tile([128, N], F8, tag=f"qk{b}{j}")
                qk8[b][j] = qt
                with tc.high_priority():
                    if b == 0 or j % 2 == 0:
                        nc.scalar.activation(out=qt, in_=pj,
                                             func=AF.Identity, bias=qkb_t[j])
                    else:
                        nc.vector.tensor_scalar_add(qt, pj, qkb_t[j])

            vtfb = vfpool.tile([128, NMC, NH, D + 1], BF16, tag="vtf")
            vtf[b] = vtfb
            nc.vector.memset(vtfb[:, :, :, D:D + 1], 1.0)
            for mcp in range(NMCP):
                pv = ps.tile([128, 2, C], F32, tag="pav", bufs=4)
                for u in range(2):
                    mc = 2 * mcp + u
                    for c in range(NCH):
                        nc.tensor.matmul(
                            out=pv[:, u, :],
                            lhsT=h8[b][:, c, mc * 128:(mc + 1) * 128],
                            rhs=wv8_t[:, c, :],
                            start=(c == 0), stop=(c == NCH - 1))
                src = pv.rearrange("p u (h f) -> p u h f", h=NH)
                with tc.high_priority():
                    if mcp % 2 == 0:
                        nc.scalar.activation(
                            out=vtfb[:, 2 * mcp:2 * mcp + 2, :, 0:D], in_=src,
                            func=AF.Copy)
                    else:
                        nc.vector.tensor_copy(
                            out=vtfb[:, 2 * mcp:2 * mcp + 2, :, 0:D], in_=src)

        emit_qkv(0)
        emit_qkv(1)

        # ---- phase 3: attention ----
        # av65[b][h]: [65, N] f32r; rows 0-63 = av (normalized in place by
        # Pool), row 64 = softmax sums (DMA'd out for the recip chain).
        av65 = [[None] * NH for _ in range(BL)]
        for b in range(BL):
            for h in range(NH):
                t = avpool.tile([D + 1, N], F32R, tag=f"av{b}{h}")
                av65[b][h] = t

        def emit_att_pair(b, nh2, pr):
            """Heads (2*pr, 2*pr+1) over n-slice nh2."""
            sl = slice(nh2 * 512, (nh2 + 1) * 512)
            jq, jk = pr, 2 + pr
            pav0 = ps.tile([D + 1, 512], F32, tag="pav", bufs=4)
            pav1 = ps.tile([D + 1, 512], F32, tag="pav", bufs=4)
            pav = [pav0, pav1]
            for mcp in range(NMCP):
                # granule (mcp, u): [128, 2(head-of-pair), 512] so the two
                # S MMs alternate row strips and run concurrently
                et = [None, None]
                for u in range(2):
                    mc = 2 * mcp + u
                    spu = ps.tile([128, 2, 512], F32, tag="sp", bufs=2)
                    for i in range(2):
                        rp = slice(64 * i, 64 * i + 64)
                        nc.tensor.matmul(
                            out=spu[:, i, :],
                            lhsT=qk8[b][jk][rp, mc * 128:(mc + 1) * 128],
                            rhs=qk8[b][jq][rp, sl],
                            start=True, stop=True,
                            tile_position=(64 * i, 0))
                    etu = etpool.tile([128, 2, 512], BF16, tag="et")
                    et[u] = etu
                    ha, hb = (slice(0, 256), slice(256, 512)) if u == 0 else                              (slice(256, 512), slice(0, 256))
                    nc.scalar.activation(out=etu[:, :, ha], in_=spu[:, :, ha],
                                         func=AF.Exp,
                                         scale=ATT_SCALE, bias=expb)
                    pi = etu.bitcast(I16)
                    nc.vector.tensor_scalar(out=pi[:, :, hb], in0=spu[:, :, hb],
                                            scalar1=SCH_MUL, scalar2=SCH_ADD,
                                            op0=OP.mult, op1=OP.add)
                for u in range(2):
                    mc = 2 * mcp + u
                    for i in range(2):
                        h = 2 * pr + i
                        nc.tensor.matmul(
                            out=pav[i],
                            lhsT=vtf[b][:, mc, h, :],
                            rhs=et[u][:, i, :],
                            start=(mcp == 0 and u == 0),
                            stop=(mcp == NMCP - 1 and u == 1),
                            skip_group_check=True)
            # tail: evacuate pav -> av65 slice; sums row -> DRAM
            hp_ctx = tc.high_priority()
            hp_ctx.__enter__()
            for i in range(2):
                h = 2 * pr + i
                dst = av65[b][h][:, sl]
                if (b + nh2 + i) % 2 == 0:
                    nc.scalar.activation(out=dst, in_=pav[i], func=AF.Copy)
                else:
                    nc.vector.tensor_copy(out=dst, in_=pav[i])
                nc.sync.dma_start(
                    out=sums_dram[b, nh2, h, :].rearrange("(o e) -> o e", o=1),
                    in_=av65[b][h][D:D + 1, sl].bitcast(F32))
            # recip chain for this pair: [2,512] -> [64,16] -> recip -> back
            rsh = sspool.tile([64, 16], F32, tag="rsh")
            nc.sync.dma_start(
                out=rsh,
                in_=sums_dram[b, nh2, 2 * pr:2 * pr + 2, :].rearrange(
                    "h (p e) -> (h p) e", e=16))
            rec = sspool.tile([64, 16], F32, tag="rec")
            nc.vector.reciprocal(out=rec, in_=rsh)
            nc.sync.dma_start(
                out=recip_dram[b, nh2, 2 * pr:2 * pr + 2, :].rearrange(
                    "h (p e) -> (h p) e", e=16),
                in_=rec)
            bc = bcpool.tile([D, 2, 512], F32, tag="bc")
            for i in range(2):
                h = 2 * pr + i
                nc.sync.dma_start(out=bc[:, i, :],
                                  in_=bcast_ap(recip_dram[b, nh2, h, :], D))
                nc.gpsimd.tensor_tensor(out=av65[b][h][0:D, sl],
                                        in0=av65[b][h][0:D, sl].bitcast(F32),
                                        in1=bc[:, i, :], op=OP.mult)
            hp_ctx.__exit__(None, None, None)

        def emit_proj(b, nh2):
            sl = slice(nh2 * 512, (nh2 + 1) * 512)
            for c in range(NCH):
                pp = ps.tile([128, 512], F32, tag="pav", bufs=4)
                for h in range(NH):
                    nc.tensor.matmul(
                        out=pp,
                        lhsT=wpT_t[h][:, c * 128:(c + 1) * 128],
                        rhs=av65[b][h][0:D, sl],
                        start=(h == 0), stop=(h == NH - 1))
                ot = outpool.tile([128, 512], F32, tag="ot")
                nc.vector.scalar_tensor_tensor(
                    out=ot, in0=pp, scalar=pbv_t[c],
                    in1=xt[b][:, c, sl].bitcast(F32), op0=OP.add, op1=OP.add)
                nc.sync.dma_start(out=out[b, c * 128:(c + 1) * 128, sl], in_=ot)

        warm_mm(3)
        for pr in range(2):
            emit_att_pair(0, 0, pr)
        for pr in range(2):
            emit_att_pair(0, 1, pr)
        emit_proj(0, 0)
        for pr in range(2):
            emit_att_pair(1, 0, pr)
        emit_proj(0, 1)
        for pr in range(2):
            emit_att_pair(1, 1, pr)
        emit_proj(1, 0)
        warm_mm(10)
        emit_proj(1, 1)

    nc.finalize()
    return nc


def _host_prep(x, gn_gamma, gn_beta, qkv_w, qkv_b, proj_w, proj_b):
    import ml_dtypes
    NPF8 = ml_dtypes.float8_e4m3
    x = np.ascontiguousarray(np.asarray(x, dtype=np.float32)).reshape(B, C, N)
    qkv_w = np.asarray(qkv_w, dtype=np.float32)
    proj_w = np.asarray(proj_w, dtype=np.float32)
    qkv_b = np.asarray(qkv_b, dtype=np.float32)
    proj_b = np.asarray(proj_b, dtype=np.float32)

    # wqk8[p, c, j, o] = qkv_w[j*128 + o, c*128 + p]  (q rows 0:256, k 256:512)
    wqk = qkv_w[:512]                      # [512, 256]
    wqk8 = np.transpose(wqk.reshape(4, 128, NCH, 128), (3, 2, 0, 1))
    wqk8 = np.ascontiguousarray(wqk8)      # [128, NCH, 4, 128]
    qkb = qkv_b[:512].reshape(4, 128).astype(np.float32)

    wv = qkv_w[2 * C:]                     # [C, C] natural
    wv8 = np.zeros((128, NCH, C), dtype=np.float32)
    for i in range(NCH):
        wv8[:, i, :] = wv[:, i * 128:(i + 1) * 128].T

    wpT4 = np.ascontiguousarray(proj_w.T.reshape(NH, D, C))
    bv = qkv_b[2 * C:]
    pbv = (proj_b + proj_w @ bv).astype(np.float32)

    sel = np.zeros((NCH, 128, G), np.float32)
    for c in range(NCH):
        for p in range(128):
            sel[c, p, (c * 128 + p) // GS] = 1.0
    sel_exp = np.zeros((G, NCH, 128), np.float32)
    for c in range(NCH):
        for p in range(128):
            sel_exp[(c * 128 + p) // GS, c, p] = 1.0

    shared = {
        "wzd": np.full((32, 1024), 0.375, dtype=NPF8),
        "wqk8": wqk8.astype(NPF8), "wv8": wv8.astype(NPF8),
        "wpT4": wpT4, "qkb": qkb, "pbv": pbv,
        "gamma": np.ascontiguousarray(np.asarray(gn_gamma, dtype=np.float32)),
        "beta": np.ascontiguousarray(np.asarray(gn_beta, dtype=np.float32)),
        "sel": sel, "sel_exp": sel_exp,
    }
    in_maps = []
    for i in range(NCORES):
        m = dict(shared)
        m["x"] = np.ascontiguousarray(x[i * BL:(i + 1) * BL])
        in_maps.append(m)
    return in_maps


def _get_nc(reps=1):
    key = f"nc{reps}"
    if key not in _CACHE:
        _CACHE[key] = _build_nc(reps)
    return _CACHE[key]


def _pjrt_callable(nc):
    """Build the sharded jitted callable once (mirrors bass2jax.run_bass_via_pjrt)."""
    import jax
    from jax.sharding import Mesh, PartitionSpec, NamedSharding
    from jax.experimental.shard_map import shard_map
    from concourse import bass2jax, mybir as mb

    bass2jax.install_neuronx_cc_hook()
    partition_name = nc.partition_id_tensor.name if nc.partition_id_tensor else None
    in_names, out_names, out_avals, zero_outs = [], [], [], []
    for alloc in nc.m.functions[0].allocations:
        if not isinstance(alloc, mb.MemoryLocationSet):
            continue
        name = alloc.memorylocations[0].name
        if alloc.kind == "ExternalInput":
            if name != partition_name:
                in_names.append(name)
        elif alloc.kind == "ExternalOutput":
            out_names.append(name)
            out_avals.append(jax.core.ShapedArray(
                tuple(alloc.tensor_shape), mb.dt.np(alloc.dtype)))
            zero_outs.append(np.zeros(tuple(alloc.tensor_shape), mb.dt.np(alloc.dtype)))
    n_params = len(in_names)
    all_in_names = list(in_names) + list(out_names)
    if partition_name is not None:
        all_in_names.append(partition_name)

    def _body(*args):
        operands = list(args)
        if partition_name is not None:
            operands.append(bass2jax.partition_id_tensor())
        outs = bass2jax._bass_exec_p.bind(
            *operands,
            out_avals=tuple(out_avals),
            in_names=tuple(all_in_names),
            out_names=tuple(out_names),
            lowering_input_output_aliases=(),
            sim_require_finite=True,
            sim_require_nnan=True,
            nc=nc,
        )
        return tuple(outs)

    devices = jax.devices()[:NCORES]
    mesh = Mesh(np.asarray(devices), ("core",))
    nspec = n_params + len(out_names)
    sharded = jax.jit(
        shard_map(_body, mesh=mesh,
                  in_specs=(PartitionSpec("core"),) * nspec,
                  out_specs=(PartitionSpec("core"),) * len(out_names),
                  check_rep=False),
        keep_unused=True)
    return sharded, in_names, out_names, zero_outs, mesh


def run(inputs, iters=1, reps=1):
    """Run on HW via PJRT. Returns (out, dispatch wall times list)."""
    import jax, time
    from jax.sharding import NamedSharding, PartitionSpec
    nc = _get_nc(reps)
    in_maps = _host_prep(**inputs)
    ckey = f"callable{reps}"
    if ckey not in _CACHE:
        _CACHE[ckey] = _pjrt_callable(nc)
    sharded, in_names, out_names, zero_outs, mesh = _CACHE[ckey]

    concat_in = [np.concatenate([in_maps[c][n] for c in range(NCORES)], axis=0)
                 for n in in_names]
    concat_zeros = [np.zeros((NCORES * z.shape[0], *z.shape[1:]), z.dtype)
                    for z in zero_outs]
    sh = NamedSharding(mesh, PartitionSpec("core"))
    dev_in = [jax.device_put(a, sh) for a in concat_in]
    dev_zero = [jax.device_put(a, sh) for a in concat_zeros]

    out_arrs = jax.block_until_ready(sharded(*dev_in, *dev_zero))
    times = []
    for _ in range(max(0, iters - 1)):
        t0 = time.perf_counter()
        out_arrs2 = jax.block_until_ready(sharded(*dev_in, *dev_zero))
        t1 = time.perf_counter()
        times.append((t1 - t0) * 1e9)

    oi = out_names.index("out")
    out = np.asarray(out_arrs[oi]).reshape(B, C, H, W)
    return out, times


def kernel(**inputs):
    out, _ = run(inputs)
    return out


# revision 38
# speedup vs baseline: 1.0721x; 1.0721x over previous
"""AttentionBlock (GroupNorm + 4-head self-attention + proj + residual) on 8 trn2 cores.

v3: HAM-warm PE + 2-way row-tiled fp8 S matmuls + bf16 AV + fast reciprocal.

Sharding: data-parallel over batch. B=16 -> 2 batches per core. Weights replicated.

Key points vs v2:
  - All attention matmuls non-DR: S = k^T q per head with K=64 in a 64-row
    strip (tile_position (64*(h%2), 0)); adjacent MMs alternate strips so
    2 heads' S MMs run concurrently in the PE array (measured 113ns/MM).
  - exp(S/8)/16 -> bf16 on ACT (native Exp) or DVE (Schraudolph int16),
    granule [128,2,512]; AV is bf16 with lhsT = [V^T | ones] (M=65).
  - Softmax recip: sums row -> DRAM -> [64,16] reshape -> DVE recip (fast,
    partition-parallel) -> DRAM -> row-broadcast [64,512]; Pool multiplies.
  - PE pre-warm dummy matmuls during the initial x DMA + fillers during
    GroupNorm stalls keep the HAM clock-gate at K=8/8 (2.4 GHz).
  - PSUM: tag "sp" 2x[128,2,512] (4 banks) + tag "pav" 4x[65/128,512]
    (4 banks) = 8 banks, time-multiplexed across GN/QKV/attention/proj.
"""
import numpy as np
from contextlib import ExitStack

import concourse.bass as bass
import concourse.bacc as bacc
import concourse.tile as tile
from concourse import mybir
from concourse import bass_utils

F32 = mybir.dt.float32
F32R = mybir.dt.float32r
BF16 = mybir.dt.bfloat16
F8 = mybir.dt.float8e4
I32 = mybir.dt.int32
I16 = mybir.dt.int16
AF = mybir.ActivationFunctionType
OP = mybir.AluOpType

B, C, H, W = 16, 256, 32, 32
N = H * W            # 1024
NH, D = 4, 64
G, GS = 32, 8        # groups, channels per group
EPS = 1e-5
NCORES = 8
BL = B // NCORES     # 2 batches per core
NCH = C // 128       # 2 channel chunks
NMC = N // 128       # 8 m-chunks
NMCP = NMC // 2      # 4 m-chunk pairs
INV_GSZ = 1.0 / (GS * N)  # 1/8192
ATT_SCALE = 0.125         # 1/sqrt(64)
LN16 = float(np.log(16.0))
# Schraudolph exp: bitcast(int16(A*x + B)) as bf16 ~ exp(x); x = S*0.125 - ln16
A_SCH = (2 ** 7) / np.log(2.0)
SCH_MUL = float(A_SCH * ATT_SCALE)
SCH_ADD = float(127 * 2 ** 7 - 366000.0 / 2 ** 16 - A_SCH * LN16)

_CACHE = {}


def _build_nc(reps=1):
    nc = bacc.Bacc()
    x = nc.declare_dram_parameter("x", [BL, C, N], F32R, isOutput=False)
    wqk8 = nc.declare_dram_parameter("wqk8", [128, NCH, 4, 128], F8, isOutput=False)
    wv8 = nc.declare_dram_parameter("wv8", [128, NCH, C], F8, isOutput=False)
    wpT4 = nc.declare_dram_parameter("wpT4", [NH, D, C], F32R, isOutput=False)
    qkb = nc.declare_dram_parameter("qkb", [4, 128], F32, isOutput=False)
    pbv = nc.declare_dram_parameter("pbv", [C], F32, isOutput=False)
    gamma = nc.declare_dram_parameter("gamma", [C], F32, isOutput=False)
    beta = nc.declare_dram_parameter("beta", [C], F32, isOutput=False)
    sel = nc.declare_dram_parameter("sel", [NCH, 128, G], F32R, isOutput=False)
    sel_exp = nc.declare_dram_parameter("sel_exp", [G, NCH, 128], F32, isOutput=False)
    wzd = nc.declare_dram_parameter("wzd", [128, 1024], F8, isOutput=False)
    out = nc.declare_dram_parameter("out", [BL, C, N], F32, isOutput=True)

    sums_dram = nc.dram_tensor("sums_dram", [BL, 2, NH, 512], F32)
    recip_dram = nc.dram_tensor("recip_dram", [BL, 2, NH, 512], F32)

    def bcast_ap(dram_row_ap, parts):
        return bass.AP(tensor=dram_row_ap.tensor, offset=dram_row_ap.offset,
                       ap=[[0, parts]] + [list(d) for d in dram_row_ap.ap])

    with tile.TileContext(nc) as tc, ExitStack() as ctx:
        if reps > 1:
            ctx.enter_context(tc.For_i(0, reps, 1, hint_engines=(
                mybir.EngineType.PE, mybir.EngineType.Activation,
                mybir.EngineType.DVE, mybir.EngineType.SP,
                mybir.EngineType.Pool)))
        const = ctx.enter_context(tc.tile_pool(name="const", bufs=1))
        xpool = ctx.enter_context(tc.tile_pool(name="xpool", bufs=2))
        sqpool = ctx.enter_context(tc.tile_pool(name="sqpool", bufs=2))
        hpool = ctx.enter_context(tc.tile_pool(name="hpool", bufs=2))
        qkpool = ctx.enter_context(tc.tile_pool(name="qkpool", bufs=1))
        vfpool = ctx.enter_context(tc.tile_pool(name="vfpool", bufs=2))
        tiny = ctx.enter_context(tc.tile_pool(name="tiny", bufs=2))
        abpool = ctx.enter_context(tc.tile_pool(name="abpool", bufs=4))
        etpool = ctx.enter_context(tc.tile_pool(name="etpool", bufs=8))
        avpool = ctx.enter_context(tc.tile_pool(name="avpool", bufs=1))
        sspool = ctx.enter_context(tc.tile_pool(name="sspool", bufs=4))
        bcpool = ctx.enter_context(tc.tile_pool(name="bcpool", bufs=4))
        outpool = ctx.enter_context(tc.tile_pool(name="outpool", bufs=3))
        ps = ctx.enter_context(tc.tile_pool(name="ps", bufs=2, space="PSUM"))

        # ---- warm operand + selexp first, then x (critical path) ----
        wz = const.tile([128, 1024], F8, tag="wz")
        nc.sync.dma_start(out=wz, in_=wzd[:, :])
        selexp_t = const.tile([G, NCH * 128], F32, tag="selexp")
        nc.sync.dma_start(out=selexp_t, in_=sel_exp.rearrange("g c p -> g (c p)"))
        xt = [None] * BL
        for b in range(BL):
            t = xpool.tile([128, NCH, N], F32R, tag="xt")
            nc.sync.dma_start(out=t, in_=x[b].rearrange("(c p) n -> p c n", p=128))
            xt[b] = t
        sel_t, gam_t, bet_t, pbv_t = [], [], [], []
        for c in range(NCH):
            t = const.tile([128, G], F32R, tag=f"sel{c}")
            nc.sync.dma_start(out=t, in_=sel[c, :, :])
            sel_t.append(t)
            t = const.tile([128, 1], F32, tag=f"gam{c}")
            nc.sync.dma_start(out=t, in_=gamma[c * 128:(c + 1) * 128].rearrange("(p o) -> p o", o=1))
            gam_t.append(t)
            t = const.tile([128, 1], F32, tag=f"bet{c}")
            nc.sync.dma_start(out=t, in_=beta[c * 128:(c + 1) * 128].rearrange("(p o) -> p o", o=1))
            bet_t.append(t)
            t = const.tile([128, 1], F32, tag=f"pbv{c}")
            nc.sync.dma_start(out=t, in_=pbv[c * 128:(c + 1) * 128].rearrange("(p o) -> p o", o=1))
            pbv_t.append(t)
        wqk8_t = const.tile([128, NCH, 4, 128], F8, tag="wqk8")
        nc.sync.dma_start(out=wqk8_t, in_=wqk8[:, :, :, :])
        wv8_t = const.tile([128, NCH, C], F8, tag="wv8")
        nc.sync.dma_start(out=wv8_t, in_=wv8[:, :, :])
        wpT_t = []
        for h in range(NH):
            t = const.tile([D, C], F32R, tag=f"wpT{h}")
            nc.sync.dma_start(out=t, in_=wpT4[h, :, :])
            wpT_t.append(t)
        qkb_t = []
        for j in range(4):
            t = const.tile([128, 1], F32, tag=f"qkb{j}")
            nc.sync.dma_start(out=t, in_=qkb[j, :].rearrange("(p o) -> p o", o=1))
            qkb_t.append(t)
        expb = const.tile([128, 1], F32, tag="expb")
        nc.vector.memset(expb, -LN16)
        scr1 = const.tile([1, 1], F32, tag="scr1")
        nc.scalar.activation(out=scr1, in_=expb[0:1, :], func=AF.Exp)
        def warm_mm(n, tag="pav"):
            for _ in range(n):
                pw = ps.tile([128, 512], F32, tag=tag, bufs=4)
                nc.tensor.matmul(out=pw, lhsT=wz[:, 0:128],
                                 rhs=wz[:, 128:640],
                                 start=True, stop=True, tile_position=(0, 0))

        warm_mm(20)

        # ---- phase 1: groupnorm -> h8 (fp8 normalized x) ----
        # bn_stats on x directly: per-partition (count, mean, count*var) for
        # even/odd halves of each 512-slice; combine to per-partition
        # (sum-ish, sumsq-ish), then one tiny sel matmul per chunk reduces
        # the 8-channel groups. stb cols as before; INV scale = 1/32.
        stb_all = [None] * BL

        def emit_gn_stats(b):
            sbn = tiny.tile([128, NCH, 2, 6], F32, tag="sbn")
            for c in range(NCH):
                for n2 in range(2):
                    nc.vector.bn_stats(
                        out=sbn[:, c, n2, :],
                        in_=xt[b][:, c, n2 * 512:(n2 + 1) * 512].bitcast(F32))
            sfx = tiny.tile([128, NCH, 2, 3], F32, tag="sfx")
            nc.vector.tensor_tensor(out=sfx[:, :, :, 0], in0=sbn[:, :, :, 1],
                                    in1=sbn[:, :, :, 4], op=OP.add)
            nc.vector.tensor_tensor(out=sfx[:, :, :, 1], in0=sbn[:, :, :, 2],
                                    in1=sbn[:, :, :, 5], op=OP.add)
            nc.vector.tensor_tensor(out=sbn[:, :, :, 0], in0=sbn[:, :, :, 1],
                                    in1=sbn[:, :, :, 1], op=OP.mult)
            nc.vector.tensor_tensor(out=sbn[:, :, :, 3], in0=sbn[:, :, :, 4],
                                    in1=sbn[:, :, :, 4], op=OP.mult)
            nc.vector.tensor_tensor(out=sfx[:, :, :, 2], in0=sbn[:, :, :, 0],
                                    in1=sbn[:, :, :, 3], op=OP.add)
            rhsb = tiny.tile([128, NCH, 2], F32, tag="rhsb")
            nc.vector.tensor_tensor(out=rhsb[:, :, 0], in0=sfx[:, :, 0, 0],
                                    in1=sfx[:, :, 1, 0], op=OP.add)
            nc.vector.tensor_tensor(out=sfx[:, :, 0, 1], in0=sfx[:, :, 0, 1],
                                    in1=sfx[:, :, 1, 1], op=OP.add)
            nc.vector.tensor_tensor(out=sfx[:, :, 0, 2], in0=sfx[:, :, 0, 2],
                                    in1=sfx[:, :, 1, 2], op=OP.add)
            nc.vector.tensor_scalar_mul(sfx[:, :, 0, 1], sfx[:, :, 0, 1],
                                        1.0 / 256.0)
            nc.vector.tensor_tensor(out=rhsb[:, :, 1], in0=sfx[:, :, 0, 1],
                                    in1=sfx[:, :, 0, 2], op=OP.add)
            stat_ps = ps.tile([G, 2], F32, tag="pav", bufs=4)
            for c in range(NCH):
                nc.tensor.matmul(out=stat_ps, lhsT=sel_t[c].bitcast(F32),
                                 rhs=rhsb[:, c, :],
                                 start=(c == 0), stop=(c == NCH - 1))
            # stats cols: 0=s 1=q 2=mean 3=rstd 4=msq 5=m2 6=var 7=lnv
            stb = tiny.tile([G, 8], F32, tag="stats")
            nc.vector.tensor_copy(out=stb[:, 0:2], in_=stat_ps)
            nc.vector.tensor_scalar_mul(stb[:, 2:3], stb[:, 0:1], 1.0 / 32.0)
            nc.vector.tensor_scalar_mul(stb[:, 4:5], stb[:, 1:2], 1.0 / 32.0)
            nc.vector.tensor_tensor(out=stb[:, 5:6], in0=stb[:, 2:3], in1=stb[:, 2:3], op=OP.mult)
            nc.vector.tensor_tensor(out=stb[:, 6:7], in0=stb[:, 4:5], in1=stb[:, 5:6], op=OP.subtract)
            # rstd via bit-trick rsqrt + 1 Newton step (all DVE)
            nc.vector.tensor_scalar_add(stb[:, 7:8], stb[:, 6:7], EPS)
            i32v = stb.bitcast(I32)
            nc.vector.tensor_scalar(out=i32v[:, 0:1], in0=i32v[:, 7:8],
                                    scalar1=1, scalar2=-1,
                                    op0=OP.logical_shift_right, op1=OP.bitwise_xor)
            nc.vector.tensor_scalar(out=i32v[:, 1:2], in0=i32v[:, 0:1],
                                    scalar1=0x5f3759e0, scalar2=None, op0=OP.add)
            nc.vector.tensor_tensor(out=stb[:, 0:1], in0=stb[:, 7:8], in1=stb[:, 1:2], op=OP.mult)
            nc.vector.tensor_tensor(out=stb[:, 0:1], in0=stb[:, 0:1], in1=stb[:, 1:2], op=OP.mult)
            nc.vector.tensor_scalar(out=stb[:, 0:1], in0=stb[:, 0:1],
                                    scalar1=-0.5, scalar2=1.5,
                                    op0=OP.mult, op1=OP.add)
            nc.vector.tensor_tensor(out=stb[:, 3:4], in0=stb[:, 1:2], in1=stb[:, 0:1], op=OP.mult)
            stb_all[b] = stb

        h8 = [None] * BL

        def emit_gn_h8(b):
            stb = stb_all[b]
            h8b = hpool.tile([128, NCH, N], F8, tag="h8")
            h8[b] = h8b
            for c in range(NCH):
                e_ps = ps.tile([128, 2], F32, tag="pav", bufs=4)
                nc.tensor.matmul(
                    out=e_ps, lhsT=selexp_t[:, c * 128:(c + 1) * 128],
                    rhs=stb[:, 2:4], start=True, stop=True)
                ab = abpool.tile([128, 3], F32, tag="ab")
                abe = abpool.tile([128, 2], F32, tag="abe")
                nc.vector.tensor_copy(out=abe, in_=e_ps)
                nc.vector.tensor_tensor(out=ab[:, 0:1], in0=abe[:, 1:2], in1=gam_t[c], op=OP.mult)
                nc.vector.tensor_tensor(out=ab[:, 2:3], in0=abe[:, 0:1], in1=ab[:, 0:1], op=OP.mult)
                nc.vector.tensor_tensor(out=ab[:, 1:2], in0=bet_t[c], in1=ab[:, 2:3], op=OP.subtract)
                nc.gpsimd.tensor_scalar(out=h8b[:, c, :], in0=xt[b][:, c, :].bitcast(F32),
                                         scalar1=ab[:, 0:1], scalar2=ab[:, 1:2],
                                         op0=OP.mult, op1=OP.add)

        emit_gn_stats(0)
        warm_mm(6)
        emit_gn_h8(0)
        warm_mm(6)
        emit_gn_stats(1)
        emit_gn_h8(1)
        warm_mm(6)

        # ---- phase 2: qkv ----
        # qk8[b][j]: [128, N] fp8; j=0: q heads01, j=1: q heads23, 2/3: k
        qk8 = [[None] * 4 for _ in range(BL)]
        vtf = [None] * BL

        def emit_qkv(b):
            for j in range(4):
                qt = qkpool.tile([128, N], F8, tag=f"qk{b}{j}")
                qk8[b][j] = qt
                for nh2 in range(2):
                    sl = slice(nh2 * 512, (nh2 + 1) * 512)
                    pj = ps.tile([128, 512], F32, tag="sp", bufs=4)
                    for c in range(NCH):
                        nc.tensor.matmul(
                            out=pj,
                            lhsT=wqk8_t[:, c, j, :],
                            rhs=h8[b][:, c, sl],
                            start=(c == 0), stop=(c == NCH - 1))
                    with tc.high_priority():
                        if b == 0 or j % 2 == 0:
                            nc.scalar.activation(out=qt[:, sl], in_=pj,
                                                 func=AF.Identity, bias=qkb_t[j])
                        else:
                            nc.vector.tensor_scalar_add(qt[:, sl], pj, qkb_t[j])

            vtfb = vfpool.tile([128, NMC, NH, D + 1], BF16, tag="vtf")
            vtf[b] = vtfb
            nc.vector.memset(vtfb[:, :, :, D:D + 1], 1.0)
            for mcp in range(NMCP):
                pv = ps.tile([128, 2, C], F32, tag="pav", bufs=4)
                for u in range(2):
                    mc = 2 * mcp + u
                    for c in range(NCH):
                        nc.tensor.matmul(
                            out=pv[:, u, :],
                            lhsT=h8[b][:, c, mc * 128:(mc + 1) * 128],
                            rhs=wv8_t[:, c, :],
                            start=(c == 0), stop=(c == NCH - 1))
                src = pv.rearrange("p u (h f) -> p u h f", h=NH)
                with tc.high_priority():
                    if mcp % 2 == 0:
                        nc.scalar.activation(
                            out=vtfb[:, 2 * mcp:2 * mcp + 2, :, 0:D], in_=src,
                            func=AF.Copy)
                    else:
                        nc.vector.tensor_copy(
                            out=vtfb[:, 2 * mcp:2 * mcp + 2, :, 0:D], in_=src)

        emit_qkv(0)
        emit_qkv(1)

        # ---- phase 3: attention ----
        # av65[b][h]: [65, N] f32r; rows 0-63 = av (normalized in place by
        # Pool), row 64 = softmax sums (DMA'd out for the recip chain).
        av65 = [[None] * NH for _ in range(BL)]
        for b in range(BL):
            for h in range(NH):
                t = avpool.tile([D + 1, N], F32R, tag=f"av{b}{h}")
                av65[b][h] = t

        def emit_att_pair(b, nh2, pr):
            """Heads (2*pr, 2*pr+1) over n-slice nh2."""
            sl = slice(nh2 * 512, (nh2 + 1) * 512)
            jq, jk = pr, 2 + pr
            pav0 = ps.tile([D + 1, 512], F32, tag="pav", bufs=4)
            pav1 = ps.tile([D + 1, 512], F32, tag="pav", bufs=4)
            pav = [pav0, pav1]
            for mcp in range(NMCP):
                # granule (mcp, u): [128, 2(head-of-pair), 512] so the two
                # S MMs alternate row strips and run concurrently
                et = [None, None]
                for u in range(2):
                    mc = 2 * mcp + u
                    # per-head 1-bank score tiles: 4 rotation slots instead
                    # of 2 pair-slots -> PE can run further ahead of exp
                    sp0 = ps.tile([128, 512], F32, tag="sp", bufs=4)
                    sp1 = ps.tile([128, 512], F32, tag="sp", bufs=4)
                    sph = [sp0, sp1]
                    for i in range(2):
                        rp = slice(64 * i, 64 * i + 64)
                        nc.tensor.matmul(
                            out=sph[i],
                            lhsT=qk8[b][jk][rp, mc * 128:(mc + 1) * 128],
                            rhs=qk8[b][jq][rp, sl],
                            start=True, stop=True,
                            tile_position=(64 * i, 0))
                    # one et tile per head: ACT and DVE touch disjoint
                    # PSUM banks and disjoint SBUF tiles -> truly parallel
                    eta = etpool.tile([128, 512], BF16, tag="et")
                    etd = etpool.tile([128, 512], BF16, tag="et")
                    ia = (mcp + u) % 2
                    idv = 1 - ia
                    nc.scalar.activation(out=eta, in_=sph[ia],
                                         func=AF.Exp,
                                         scale=ATT_SCALE, bias=expb)
                    nc.vector.tensor_scalar(out=etd.bitcast(I16),
                                            in0=sph[idv],
                                            scalar1=SCH_MUL, scalar2=SCH_ADD,
                                            op0=OP.mult, op1=OP.add)
                    eth = [None, None]
                    eth[ia] = eta
                    eth[idv] = etd
                    et[u] = eth
                for u in range(2):
                    mc = 2 * mcp + u
                    for i in range(2):
                        h = 2 * pr + i
                        nc.tensor.matmul(
                            out=pav[i],
                            lhsT=vtf[b][:, mc, h, :],
                            rhs=et[u][i],
                            start=(mcp == 0 and u == 0),
                            stop=(mcp == NMCP - 1 and u == 1),
                            skip_group_check=True)
            # tail: evacuate pav -> av65 slice; sums row -> DRAM
            for i in range(2):
                h = 2 * pr + i
                dst = av65[b][h][:, sl]
                if (b + nh2 + i) % 2 == 0:
                    nc.scalar.activation(out=dst, in_=pav[i], func=AF.Copy)
                else:
                    nc.vector.tensor_copy(out=dst, in_=pav[i])
            hp_ctx = tc.high_priority()
            hp_ctx.__enter__()
            for i in range(2):
                h = 2 * pr + i
                nc.sync.dma_start(
                    out=sums_dram[b, nh2, h, :].rearrange("(o e) -> o e", o=1),
                    in_=av65[b][h][D:D + 1, sl].bitcast(F32))
            # recip chain for this pair: [2,512] -> [64,16] -> recip -> back
            rsh = sspool.tile([64, 16], F32, tag="rsh")
            nc.sync.dma_start(
                out=rsh,
                in_=sums_dram[b, nh2, 2 * pr:2 * pr + 2, :].rearrange(
                    "h (p e) -> (h p) e", e=16))
            rec = sspool.tile([64, 16], F32, tag="rec")
            nc.vector.reciprocal(out=rec, in_=rsh)
            nc.sync.dma_start(
                out=recip_dram[b, nh2, 2 * pr:2 * pr + 2, :].rearrange(
                    "h (p e) -> (h p) e", e=16),
                in_=rec)
            bc = bcpool.tile([D, 2, 512], F32, tag="bc")
            for i in range(2):
                h = 2 * pr + i
                nc.sync.dma_start(out=bc[:, i, :],
                                  in_=bcast_ap(recip_dram[b, nh2, h, :], D))
                nc.gpsimd.tensor_tensor(out=av65[b][h][0:D, sl],
                                        in0=av65[b][h][0:D, sl].bitcast(F32),
                                        in1=bc[:, i, :], op=OP.mult)
            hp_ctx.__exit__(None, None, None)

        def emit_proj(b, nh2):
            sl = slice(nh2 * 512, (nh2 + 1) * 512)
            for c in range(NCH):
                pp = ps.tile([128, 512], F32, tag="pav", bufs=4)
                for h in range(NH):
                    nc.tensor.matmul(
                        out=pp,
                        lhsT=wpT_t[h][:, c * 128:(c + 1) * 128],
                        rhs=av65[b][h][0:D, sl],
                        start=(h == 0), stop=(h == NH - 1))
                ot = outpool.tile([128, 512], F32, tag="ot")
                nc.vector.scalar_tensor_tensor(
                    out=ot, in0=pp, scalar=pbv_t[c],
                    in1=xt[b][:, c, sl].bitcast(F32), op0=OP.add, op1=OP.add)
                nc.sync.dma_start(out=out[b, c * 128:(c + 1) * 128, sl], in_=ot)

        warm_mm(3)
        for pr in range(2):
            emit_att_pair(0, 0, pr)
        warm_mm(4, tag="sp")
        for pr in range(2):
            emit_att_pair(0, 1, pr)
        warm_mm(4, tag="sp")
        emit_proj(0, 0)
        for pr in range(2):
            emit_att_pair(1, 0, pr)
        warm_mm(4, tag="sp")
        emit_proj(0, 1)
        for pr in range(2):
            emit_att_pair(1, 1, pr)
        warm_mm(4, tag="sp")
        emit_proj(1, 0)
        warm_mm(16, tag="sp")
        emit_proj(1, 1)

    nc.finalize()
    return nc


def _host_prep(x, gn_gamma, gn_beta, qkv_w, qkv_b, proj_w, proj_b):
    import ml_dtypes
    NPF8 = ml_dtypes.float8_e4m3
    x = np.ascontiguousarray(np.asarray(x, dtype=np.float32)).reshape(B, C, N)
    qkv_w = np.asarray(qkv_w, dtype=np.float32)
    proj_w = np.asarray(proj_w, dtype=np.float32)
    qkv_b = np.asarray(qkv_b, dtype=np.float32)
    proj_b = np.asarray(proj_b, dtype=np.float32)

    # wqk8[p, c, j, o] = qkv_w[j*128 + o, c*128 + p]  (q rows 0:256, k 256:512)
    wqk = qkv_w[:512]                      # [512, 256]
    wqk8 = np.transpose(wqk.reshape(4, 128, NCH, 128), (3, 2, 0, 1))
    wqk8 = np.ascontiguousarray(wqk8)      # [128, NCH, 4, 128]
    qkb = qkv_b[:512].reshape(4, 128).astype(np.float32)

    wv = qkv_w[2 * C:]                     # [C, C] natural
    wv8 = np.zeros((128, NCH, C), dtype=np.float32)
    for i in range(NCH):
        wv8[:, i, :] = wv[:, i * 128:(i + 1) * 128].T

    wpT4 = np.ascontiguousarray(proj_w.T.reshape(NH, D, C))
    bv = qkv_b[2 * C:]
    pbv = (proj_b + proj_w @ bv).astype(np.float32)

    sel = np.zeros((NCH, 128, G), np.float32)
    for c in range(NCH):
        for p in range(128):
            sel[c, p, (c * 128 + p) // GS] = 1.0
    sel_exp = np.zeros((G, NCH, 128), np.float32)
    for c in range(NCH):
        for p in range(128):
            sel_exp[(c * 128 + p) // GS, c, p] = 1.0

    shared = {
        "wzd": np.full((128, 1024), 0.375, dtype=NPF8),
        "wqk8": wqk8.astype(NPF8), "wv8": wv8.astype(NPF8),
        "wpT4": wpT4, "qkb": qkb, "pbv": pbv,
        "gamma": np.ascontiguousarray(np.asarray(gn_gamma, dtype=np.float32)),
        "beta": np.ascontiguousarray(np.asarray(gn_beta, dtype=np.float32)),
        "sel": sel, "sel_exp": sel_exp,
    }
    in_maps = []
    for i in range(NCORES):
        m = dict(shared)
        m["x"] = np.ascontiguousarray(x[i * BL:(i + 1) * BL])
        in_maps.append(m)
    return in_maps


def _get_nc(reps=1):
    key = f"nc{reps}"
    if key not in _CACHE:
        _CACHE[key] = _build_nc(reps)
    return _CACHE[key]


def _pjrt_callable(nc):
    """Build the sharded jitted callable once (mirrors bass2jax.run_bass_via_pjrt)."""
    import jax
    from jax.sharding import Mesh, PartitionSpec, NamedSharding
    from jax.experimental.shard_map import shard_map
    from concourse import bass2jax, mybir as mb

    bass2jax.install_neuronx_cc_hook()
    partition_name = nc.partition_id_tensor.name if nc.partition_id_tensor else None
    in_names, out_names, out_avals, zero_outs = [], [], [], []
    for alloc in nc.m.functions[0].allocations:
        if not isinstance(alloc, mb.MemoryLocationSet):
            continue
        name = alloc.memorylocations[0].name
        if alloc.kind == "ExternalInput":
            if name != partition_name:
                in_names.append(name)
        elif alloc.kind == "ExternalOutput":
            out_names.append(name)
            out_avals.append(jax.core.ShapedArray(
                tuple(alloc.tensor_shape), mb.dt.np(alloc.dtype)))
            zero_outs.append(np.zeros(tuple(alloc.tensor_shape), mb.dt.np(alloc.dtype)))
    n_params = len(in_names)
    all_in_names = list(in_names) + list(out_names)
    if partition_name is not None:
        all_in_names.append(partition_name)

    def _body(*args):
        operands = list(args)
        if partition_name is not None:
            operands.append(bass2jax.partition_id_tensor())
        outs = bass2jax._bass_exec_p.bind(
            *operands,
            out_avals=tuple(out_avals),
            in_names=tuple(all_in_names),
            out_names=tuple(out_names),
            lowering_input_output_aliases=(),
            sim_require_finite=True,
            sim_require_nnan=True,
            nc=nc,
        )
        return tuple(outs)

    devices = jax.devices()[:NCORES]
    mesh = Mesh(np.asarray(devices), ("core",))
    nspec = n_params + len(out_names)
    sharded = jax.jit(
        shard_map(_body, mesh=mesh,
                  in_specs=(PartitionSpec("core"),) * nspec,
                  out_specs=(PartitionSpec("core"),) * len(out_names),
                  check_rep=False),
        keep_unused=True)
    return sharded, in_names, out_names, zero_outs, mesh


def run(inputs, iters=1, reps=1):
    """Run on HW via PJRT. Returns (out, dispatch wall times list)."""
    import jax, time
    from jax.sharding import NamedSharding, PartitionSpec
    nc = _get_nc(reps)
    in_maps = _host_prep(**inputs)
    ckey = f"callable{reps}"
    if ckey not in _CACHE:
        _CACHE[ckey] = _pjrt_callable(nc)
    sharded, in_names, out_names, zero_outs, mesh = _CACHE[ckey]

    concat_in = [np.concatenate([in_maps[c][n] for c in range(NCORES)], axis=0)
                 for n in in_names]
    concat_zeros = [np.zeros((NCORES * z.shape[0], *z.shape[1:]), z.dtype)
                    for z in zero_outs]
    sh = NamedSharding(mesh, PartitionSpec("core"))
    dev_in = [jax.device_put(a, sh) for a in concat_in]
    dev_zero = [jax.device_put(a, sh) for a in concat_zeros]

    out_arrs = jax.block_until_ready(sharded(*dev_in, *dev_zero))
    times = []
    for _ in range(max(0, iters - 1)):
        t0 = time.perf_counter()
        out_arrs2 = jax.block_until_ready(sharded(*dev_in, *dev_zero))
        t1 = time.perf_counter()
        times.append((t1 - t0) * 1e9)

    oi = out_names.index("out")
    out = np.asarray(out_arrs[oi]).reshape(B, C, H, W)
    return out, times


def kernel(**inputs):
    out, _ = run(inputs)
    return out
